# revision 33
# baseline (speedup 1.0000x reference)
"""GraphSAGE-style 2-layer GNN minibatch forward on 8 trn2 NeuronCores.

Data-parallel over the 1024 target nodes: each core handles 128 targets.

The host pre-expands the 2-level node tree into a per-core, per-group
feature stream laid out TRANSPOSED ([feature, slot, chunk, token]) so
the device does no gather at all: each 128-token group is one linear
~0.9 MB dma_start on the SP HWDGE ring (full HBM bandwidth). Self rows
travel in bf16, neighbor rows in fp8-e4m3 (they only enter through a
mean of 25, which washes out the quantization noise; measured
end-to-end rel err ~4e-3).

The feature stream owns the SP HWDGE ring exclusively; constants ride
the ACT ring, issued up front so they land during the stream lead-in.
The drain-time w1/b1 constants are data-gated to mid-loop so their
transfer cannot steal stream bandwidth (a dep-free dma_start gets
hoisted into the preamble by the scheduler). Only the pipeline-fill
and final groups are split into half-transfers: every transfer costs
one of the 8 shared HWDGE semaphore lanes, and too many transfers let
the issue backlog collapse into a latency-bound trickle at the tail.

Neighbor aggregation runs on the PE as identity-weight DoubleRow
matmuls (two fp8 slots summed per streamed column) accumulating in
PSUM; the [feat, token] result is exactly the lhsT layout the MLP
matmuls need, so there are no on-device transposes. The mean /S is
folded into the weight matrices on the host. The biases are folded
into the DVE epilogue (ph + replicated-bias row, then relu) instead of
rank-1 bias matmuls: a bias matmul has no data dependency on its
group, so the Tile scheduler hoisted it many groups early and the PE
then sat ~0.8 us per group on a PSUM bank WAR against a relu that had
not run yet. With every PSUM-writing matmul gated on current-group
data the hoisting is bounded and the PE never blocks. PSUM banks: 2
aggregation accumulators (+ warm-up junk), 4 MLP outputs, 2 layer-1
transposed accumulators. Dummy warm-up matmuls hold the PE's HAM
activity monitor at the 2.4 GHz p-state through the cold start and the
drain.

All shapes hardcoded; self-contained (only needs the concourse runtime
that ships with the container).
"""

import numpy as np

N_CORES = 8
N_NODES = 100000
D = 256          # feature dim
P = 128          # partitions / tokens per group
B = 1024         # total targets
S0 = 25          # layer-0 fanout
S1 = 10          # layer-1 fanout
NG = 11          # groups of 128 tokens per core at layer 1 (1408 = 11*128)
SBYTES = 2 * P * 2 + S0 * 2 * P   # 6912 stream bytes/partition: self bf16 + neigh fp8
NC0 = 128        # identity const tile columns (bf16 view of fp8 DoubleRow identity)
NCA = 1280       # layer-0 const tile columns (bf16): w0 chunks, replicated b0
NCC = 1408       # mid const tile columns (bf16): a1 selectors, ident
NCT = 1280       # tail const tile columns (bf16): w1 chunks, replicated b1

_CACHE = {}
_USE_TTR = False   # DVE square+accum in tail epilogues (breaks on HW)


def _build_program():
    import concourse.bacc as bacc
    import concourse.mybir as mybir
    import concourse.tile as tile

    F32 = mybir.dt.float32
    BF16 = mybir.dt.bfloat16
    FP8 = mybir.dt.float8e4
    I8 = mybir.dt.int8
    AF = mybir.ActivationFunctionType
    PM = mybir.MatmulPerfMode
    ALU = mybir.AluOpType

    nc = bacc.Bacc("TRN2", target_bir_lowering=False, debug=False)

    st_d = nc.dram_tensor("st", [NG, P, SBYTES], I8, kind="ExternalInput")
    cst0_d = nc.dram_tensor("cst0", [P, NC0], BF16, kind="ExternalInput")
    csta_d = nc.dram_tensor("csta", [P, NCA], BF16, kind="ExternalInput")
    cstbc_d = nc.dram_tensor("cstbc", [P, NCC], BF16, kind="ExternalInput")
    cstbt_d = nc.dram_tensor("cstbt", [P, NCT], BF16, kind="ExternalInput")
    out_d = nc.dram_tensor("out", [P, D], F32, kind="ExternalOutput")

    with tile.TileContext(nc) as tc:
        with (
            tc.tile_pool(name="consts", bufs=1) as consts,
            tc.tile_pool(name="gatp", bufs=9) as gatp,
            tc.tile_pool(name="aggp", bufs=4) as aggp,
            tc.tile_pool(name="xtp", bufs=1) as xtp,
            tc.tile_pool(name="epip", bufs=2) as epip,
            tc.tile_pool(name="aggps", bufs=2, space="PSUM") as aggps,
            tc.tile_pool(name="mmp", bufs=4, space="PSUM") as mmp,
            tc.tile_pool(name="l1ps", bufs=1, space="PSUM") as l1ps,
        ):
            # consts ride the ACT HWDGE ring (issued first, they finish
            # during the stream lead-in); the SP ring carries ONLY the
            # feature stream so its last byte lands as early as possible
            cst0 = consts.tile([P, NC0], BF16, tag="cst0")
            nc.scalar.dma_start(out=cst0[:], in_=cst0_d[:])
            csta = consts.tile([P, NCA], BF16, tag="csta")
            nc.scalar.dma_start(out=csta[:], in_=csta_d[:])
            cstbc = consts.tile([P, NCC], BF16, tag="cstbc")
            nc.scalar.dma_start(out=cstbc[:], in_=cstbc_d[:])

            pend = {}
            HALF = 2 * P * 2 + 12 * 2 * P   # self + neighbor slots 0-11

            def load_group(g):
                # only the pipeline-fill and drain-critical groups are
                # halved: every transfer costs one of the 8 shared HWDGE
                # sem lanes, and too many small transfers let the issue
                # backlog collapse at the stream tail (each issue waits
                # its lane's previous transfer completion)
                t = gatp.tile([P, SBYTES], I8, tag="gat")
                if g in (0, 1, NG - 2, NG - 1):
                    nc.sync.dma_start(out=t[:, 0:HALF], in_=st_d[g][:, 0:HALF])
                    nc.sync.dma_start(
                        out=t[:, HALF:SBYTES], in_=st_d[g][:, HALF:SBYTES]
                    )
                else:
                    nc.sync.dma_start(out=t[:], in_=st_d[g])
                return t

            pend[0] = load_group(0)
            pend[1] = load_group(1)
            pend[2] = load_group(2)
            pend[3] = load_group(3)

            id2 = cst0[:, 0:NC0].bitcast(FP8)              # [P, 2*P] fp8
            id2_dr = id2.rearrange("p (j m) -> p j m", j=2)
            id1_8 = id2[:, 0:P]                            # [P, P] fp8 identity
            w0_sb = [csta[:, c * D:(c + 1) * D] for c in range(4)]
            b0rep = csta[:, 1024:1280]                     # [P, D] bf16, b0 per row
            a1_sb = [cstbc[:, j * P:(j + 1) * P] for j in range(S1)]
            ident = cstbc[:, 1280:1408]                    # [P, P] bf16
            cstbt = consts.tile([P, NCT], BF16, tag="cstbt")
            w1_sb = [cstbt[:, c * D:(c + 1) * D] for c in range(4)]
            b1rep = cstbt[:, 1024:1280]                    # [P, D] bf16, b1 per row

            # scratch + eps on the otherwise-idle GpSimd engine so the DVE
            # sem lane carries only loop ops
            scr = consts.tile([P, D], BF16, tag="scr")
            nc.gpsimd.memset(scr[:], 0.0)
            eps = consts.tile([P, 1], F32, tag="eps")
            nc.gpsimd.memset(eps[:], 1e-30)

            def warm(n, cols=D):
                # dummy matmuls that keep the PE activity monitor busy so
                # the clock gate stays at (or ramps to) 2.4 GHz; they borrow
                # the aggregation PSUM banks (same-engine ordering only, so
                # they never add cross-engine waits in front of real MMs)
                junk = aggps.tile([P, D], F32, tag="pagg", name="junk")
                for _ in range(n):
                    nc.tensor.matmul(
                        out=junk[:, 0:cols], lhsT=scr[:, 0:P], rhs=scr[:, 0:cols],
                        start=True, stop=True,
                    )

            h1_sb = [
                consts.tile([P, D], BF16, tag=f"h1_{g}", name=f"h1_{g}")
                for g in range(NG)
            ]
            out_sb = consts.tile([P, D], F32, tag="out_sb")

            def epilogue(ph, out_t, brep, tail=False):
                # out_t = l2norm(relu(ph + brep)) per token (partition); the
                # bias add + relu run on the DVE so the ph PSUM bank frees
                # without waiting on the ACT queue; the squared-norm
                # accumulation runs on ACT in the loop (DVE slack) but on
                # the DVE in the tail (skips ACT's accumulator-read hop)
                h1a = epip.tile([P, D], BF16, tag="h1a")
                nc.vector.tensor_add(h1a[:], ph[:], brep)
                h1r = epip.tile([P, D], BF16, tag="h1r")
                nc.vector.tensor_scalar_max(h1r[:], h1a[:], 0.0)
                trash = epip.tile([P, D], BF16, tag="trash")
                n2 = epip.tile([P, 1], F32, tag="n2")
                if tail and _USE_TTR:
                    nc.vector.tensor_tensor_reduce(
                        out=trash[:], in0=h1r[:], in1=h1r[:], scale=1.0,
                        scalar=0.0, op0=ALU.mult, op1=ALU.add, accum_out=n2[:],
                    )
                else:
                    nc.scalar.activation(
                        out=trash[:], in_=h1r[:], func=AF.Square, accum_out=n2[:]
                    )
                nrm = epip.tile([P, 1], F32, tag="nrm")
                nc.scalar.activation(out=nrm[:], in_=n2[:], func=AF.Sqrt, bias=eps[:])
                rinv = epip.tile([P, 1], F32, tag="rinv")
                nc.vector.reciprocal(out=rinv[:], in_=nrm[:])
                nc.vector.tensor_scalar_mul(out_t[:], h1r[:], rinv[:])

            def mlp(ph, xts, w_sb):
                # bias is folded into the epilogue: every matmul here reads
                # current-group data, so the scheduler cannot hoist the
                # PSUM-bank-clearing start matmul ahead of the data
                for i, x in enumerate(xts):
                    nc.tensor.matmul(
                        out=ph[:], lhsT=x, rhs=w_sb[i],
                        start=(i == 0), stop=(i == 3),
                    )

            def agg_mms(nb, pagg):
                # neighbor sum on PE: 2 fp8 slots per DoubleRow matmul
                for k in range(S0 // 2):
                    nc.tensor.matmul(
                        out=pagg[:], lhsT=id2_dr,
                        rhs=nb[:, k * 2 * D:(k + 1) * 2 * D].rearrange(
                            "p (j n) -> p j n", j=2
                        ),
                        start=(k == 0), stop=False, perf_mode=PM.DoubleRow,
                    )
                nc.tensor.matmul(
                    out=pagg[:], lhsT=id1_8, rhs=nb[:, (S0 - 1) * D:S0 * D],
                    start=False, stop=True,
                )

            # layer-1 transposed aggregation accumulators + lhsT tiles
            # (one PSUM tile per feature chunk: matmul start=True clears
            # has_written bank-wide, so the chunks must not share a bank)
            agg1t = [
                l1ps.tile([P, P], F32, tag=f"agg1t{c}", name=f"agg1t{c}")
                for c in range(2)
            ]
            xts1 = [
                xtp.tile([P, P], BF16, tag=f"xt{i}", name=f"xt{i}")
                for i in range(4)
            ]

            def a1t_mms(j, stop):
                # layer-1 aggregation, transposed: agg1t[f, tgt] accumulates
                # h1[1+j].T @ a1[j] chunk-wise
                for c in range(2):
                    nc.tensor.matmul(
                        out=agg1t[c][:],
                        lhsT=h1_sb[1 + j][:, c * P:(c + 1) * P],
                        rhs=a1_sb[j],
                        start=(j == 0), stop=stop,
                    )

            # hold the PE busy through the cold start so HAM promotes the
            # clock right as group 0's data lands; the narrow matmuls at the
            # end give fine-grained coverage without overshooting past the
            # data arrival
            warm(10)
            warm(14, cols=64)

            def do_mlp(entry):
                pself, paggs, pg = entry
                ph = mmp.tile([P, D], F32, tag="ph")
                mlp(
                    ph,
                    [pself[:, 0:P], pself[:, P:2 * P],
                     paggs[:, 0:P], paggs[:, P:2 * P]],
                    w0_sb,
                )
                return ph, pg

            # ---- layer 0: 11 groups, MLP pipelined TWO groups behind the
            # aggregation. Each iteration is pinned to a scheduling band
            # via tile_wait_until (a sim-time floor the Tile scheduler
            # honors): without the bands the greedy scheduler collapses
            # the software pipeline to depth zero and parks PE consumers
            # of fresh DVE output right behind the epilogue chain, idling
            # the in-order PE queue ~1 us per group at runtime ----
            prevs = []
            for g in range(NG):
                with tc.tile_wait_until(1 + g):
                    gat = pend.pop(g)
                    if g + 4 < NG:
                        pend[g + 4] = load_group(g + 4)
                    if g == 5:
                        # gate the drain-time consts DMA on group-5 data (a
                        # dep-free dma_start gets hoisted into the preamble
                        # by the scheduler and its ACT-ring trickle then
                        # steals stream bandwidth in the critical window)
                        nc.gpsimd.tensor_copy(
                            out=cstbt[0:1, 0:1],
                            in_=gat[0:1, 0:2].bitcast(BF16),
                        )
                        nc.scalar.dma_start(out=cstbt[:], in_=cstbt_d[:])
                    if g == 0:
                        warm(8, cols=64)   # bridge early DMA-sem waits
                    if g in (1, 2):
                        warm(8, cols=64)
                    if g == 3:
                        warm(4, cols=64)
                    self_bf = gat[:, 0:2 * P * 2].bitcast(BF16)  # [P, 2*P]
                    nb = gat[:, 2 * P * 2:SBYTES].bitcast(FP8)   # [P, S0*2*P]
                    if len(prevs) == 2:
                        ph, pg = do_mlp(prevs.pop(0))
                        epilogue(ph, h1_sb[pg], b0rep)
                    pagg = aggps.tile([P, D], F32, tag="pagg")
                    agg_mms(nb, pagg)
                    aggs = aggp.tile([P, D], BF16, tag="aggs")
                    nc.vector.tensor_copy(out=aggs[:], in_=pagg[:])
                    prevs.append((self_bf, aggs, g))

            # ---- drain band: groups 9/10 MLPs, all layer-1 transposed
            # aggregation (the tiny a1t matmuls soak up the PE idle during
            # the serial tail epilogues), then the layer-1 MLP ----
            with tc.tile_wait_until(NG + 2):
                phA, pgA = do_mlp(prevs.pop(0))          # group 9
                epilogue(phA, h1_sb[pgA], b0rep)         # -> h1[9]
                phB, pgB = do_mlp(prevs.pop(0))          # group 10
                warm(10, cols=64)   # hold HAM through the drain epilogues
                epilogue(phB, h1_sb[pgB], b0rep)         # -> h1[10]
                # transpose layer-1 self (h1[0]) into lhsT layout via
                # identity-rhs matmuls, borrowing the agg1t PSUM banks
                # before their accumulation opens
                for i in range(2):
                    nc.tensor.matmul(
                        out=agg1t[i][:],
                        lhsT=h1_sb[0][:, i * P:(i + 1) * P],
                        rhs=ident, start=True, stop=True,
                    )
                    nc.vector.tensor_copy(out=xts1[i][:], in_=agg1t[i][:])
                for j in range(S1 - 1):
                    a1t_mms(j, stop=False)               # h1[1..9]
                a1t_mms(S1 - 1, stop=True)               # h1[10]

                # layer 1 tail (copies split DVE/ACT to run in parallel)
                nc.vector.tensor_copy(out=xts1[2][:], in_=agg1t[0][:])
                nc.scalar.copy(out=xts1[3][:], in_=agg1t[1][:])
                ph1 = mmp.tile([P, D], F32, tag="ph")
                mlp(ph1, [x[:] for x in xts1], w1_sb)
                # final epilogue with the last scale+store split into column
                # halves on both HWDGE rings so the second half's store
                # overlaps the first's
                h1a = epip.tile([P, D], BF16, tag="h1a")
                nc.vector.tensor_add(h1a[:], ph1[:], b1rep)
                h1r = epip.tile([P, D], BF16, tag="h1r")
                nc.vector.tensor_scalar_max(h1r[:], h1a[:], 0.0)
                trash = epip.tile([P, D], BF16, tag="trash")
                n2 = epip.tile([P, 1], F32, tag="n2")
                nc.scalar.activation(
                    out=trash[:], in_=h1r[:], func=AF.Square, accum_out=n2[:]
                )
                nrm = epip.tile([P, 1], F32, tag="nrm")
                nc.scalar.activation(out=nrm[:], in_=n2[:], func=AF.Sqrt, bias=eps[:])
                rinv = epip.tile([P, 1], F32, tag="rinv")
                nc.vector.reciprocal(out=rinv[:], in_=nrm[:])
                nc.vector.tensor_scalar_mul(out_sb[:, 0:P], h1r[:, 0:P], rinv[:])
                nc.sync.dma_start(out=out_d[:, 0:P], in_=out_sb[:, 0:P])
                nc.vector.tensor_scalar_mul(out_sb[:, P:D], h1r[:, P:D], rinv[:])
                nc.scalar.dma_start(out=out_d[:, P:D], in_=out_sb[:, P:D])

    nc.compile()
    return nc


def get_program():
    if "nc" not in _CACHE:
        _CACHE["nc"] = _build_program()
    return _CACHE["nc"]


def prepare_in_maps(features, W0, b0, W1, b1, nodes2, neigh2, neigh1):
    """Host-side sharding + expanded transposed bf16/fp8 feature stream."""
    import ml_dtypes

    BF16 = ml_dtypes.bfloat16
    FP8 = ml_dtypes.float8_e4m3

    features = np.ascontiguousarray(features, dtype=np.float32)
    featsb = features.astype(BF16)
    feats8 = features.astype(FP8)

    # identity const tile [128, 128]: the fp8 DoubleRow identity
    id2 = np.zeros((P, 2 * P), dtype=FP8)  # [p, (j, m)] DoubleRow identity
    id2[np.arange(P), np.arange(P)] = 1.0
    id2[np.arange(P), P + np.arange(P)] = 1.0
    cst0 = np.ascontiguousarray(id2.view(np.uint8).view(BF16))

    # layer-0 consts [128, 1280]: w0 chunks, replicated b0
    csta = np.zeros((P, NCA), dtype=BF16)
    w0 = np.ascontiguousarray(W0.T, dtype=np.float32).copy()
    w0[D:, :] /= S0  # fold the layer-0 neighbor mean into the weights
    csta[:, 0:1024] = (
        w0.reshape(4, P, D).transpose(1, 0, 2).reshape(P, 1024).astype(BF16)
    )
    csta[:, 1024:1280] = b0.astype(BF16)[None, :]

    # mid consts [128, 1408]: a1 selection matrices, identity
    # layer-1 aggregation: token 128*g + p (g>=1) is neighbor
    # j = 128*(g-1) + p of target j // 10
    cstbc = np.zeros((P, NCC), dtype=BF16)
    a1 = np.zeros((S1, P, P), dtype=np.float32)
    j = np.arange(P * S1)
    a1[j // P, j % P, j // S1] = 1.0
    cstbc[:, 0:1280] = a1.transpose(1, 0, 2).reshape(P, S1 * P).astype(BF16)
    cstbc[:, 1280:1408] = np.eye(P, dtype=np.float32).astype(BF16)

    # tail consts [128, 1280]: w1 chunks, replicated b1
    cstbt = np.zeros((P, NCT), dtype=BF16)
    w1 = np.ascontiguousarray(W1.T, dtype=np.float32).copy()
    w1[D:, :] /= S1
    cstbt[:, 0:1024] = (
        w1.reshape(4, P, D).transpose(1, 0, 2).reshape(P, 1024).astype(BF16)
    )
    cstbt[:, 1024:1280] = b1.astype(BF16)[None, :]

    in_maps = []
    bc = B // N_CORES  # 128 targets per core
    for c in range(N_CORES):
        nodes2_c = nodes2[c * bc:(c + 1) * bc]
        neigh2_c = neigh2[c * bc:(c + 1) * bc, :]
        nodes1_c = np.concatenate([nodes2_c, neigh2_c.reshape(-1)])
        neigh1_c = np.concatenate(
            [
                neigh1[c * bc:(c + 1) * bc, :],
                neigh1[B + c * bc * S1:B + (c + 1) * bc * S1, :],
            ],
            axis=0,
        )
        # self stream: [g, t, (c, f)] -> [g, f, c, t] bf16
        selfT = (
            featsb[nodes1_c]
            .reshape(NG, P, 2, P)
            .transpose(0, 3, 2, 1)
        )
        self_u8 = np.ascontiguousarray(selfT).view(np.uint8).reshape(NG, P, -1)
        # neighbor stream: [g, t, s, (c, f)] -> [g, f, s, c, t] fp8
        nbT = (
            feats8[neigh1_c.reshape(-1)]
            .reshape(NG, P, S0, 2, P)
            .transpose(0, 4, 2, 3, 1)
        )
        nb_u8 = np.ascontiguousarray(nbT).view(np.uint8).reshape(NG, P, -1)
        st = np.concatenate([self_u8, nb_u8], axis=2).view(np.int8)
        in_maps.append(
            {"st": st, "cst0": cst0, "csta": csta, "cstbc": cstbc, "cstbt": cstbt}
        )
    return in_maps


def kernel(features, W0, b0, W1, b1, nodes2, neigh2, neigh1, _trace=False):
    from concourse.bass_utils import run_bass_kernel_spmd

    nc = get_program()
    in_maps = prepare_in_maps(features, W0, b0, W1, b1, nodes2, neigh2, neigh1)
    kwargs = {}
    if _trace:
        import tempfile

        import ntff_shim  # noqa: F401  (registers the axon NTFF hook)

        kwargs = {"trace": True, "tmpdir": tempfile.mkdtemp(prefix="ntff_")}
    res = run_bass_kernel_spmd(nc, in_maps, list(range(N_CORES)), **kwargs)
    out = np.concatenate([res.results[c]["out"] for c in range(N_CORES)], axis=0)
    if _trace:
        _CACHE["last_result"] = res
    return out
